# revision 1
# baseline (speedup 1.0000x reference)
"""Trainium2 Bass kernel for CrossAttention.

Problem shape (hardcoded):
  latent  [8, 4096, 512], context [8, 77, 768]
  wq [512,512], wk/wv [768,512], wo [512,512], biases [512]
  out = softmax((latent@wq+bq)(context@wk+bk)^T / 8) @ (context@wv+bv) @ wo + bo

Sharding: data-parallel over batch — core b handles batch element b.

All matmuls keep the PE contraction dim (partitions) at FULL K=128 coverage:
mixing partial row-group masks between consecutive matmuls hangs/crashes TRN2
(verified empirically), so per-head operands (Dh=64, SKV=77) are stored
head-major and zero-padded up to 128 partitions. Zero rows contribute nothing
to the contraction, and a matmul's cycle cost scales with N only, so the
padding is free in PE time.

Per-core dataflow:
  one-time:
    cT   = context^T (zero-padded)     [128, 6, 77]
    kT_h = 0.125 * (wk_h^T cT + bk_h)  [128(pad 64), 8, 77]   head-major
    v    = cT^T wv + bv                [128(pad 77), 512]
  per 128-row chunk of latent:
    xT    = x^T                        [512, 128]  (PE transpose)
    qT_h  = wq_h^T xT (+bq_h)          [128(pad 64), 8, 128]  head-major
    sT_h  = kT_h^T qT_h                [77, 128]  (scaled scores^T)
    eT_h  = exp(sT_h)                  [128(pad 77), 8, 128]  (no max-sub; |s|<8)
    sums  = ones^T eT_h                [128, 4, 128]  d-major, bcast over parts
    attnT = v_h^T eT_h * 1/sums        [128, 4, 128]  d-major
    out   = attnT^T wo + bo            [128, 512]
"""

import os
import sys
from contextlib import ExitStack

import numpy as np

for _p in ("/opt/trn_rl_repo",):
    if _p not in sys.path and os.path.isdir(_p):
        sys.path.insert(0, _p)

import concourse.bass as bass  # noqa: E402
import concourse.tile as tile  # noqa: E402
from concourse import bacc, mybir  # noqa: E402
from concourse.bass_utils import run_bass_kernel_spmd  # noqa: E402
from concourse.masks import make_identity  # noqa: E402

N_CORES = 8
SQ, D, DC, SKV, H, DH = 4096, 512, 768, 77, 8, 64
F32 = mybir.dt.float32
AF = mybir.ActivationFunctionType

# Matmul operand dtype: 'bf16' (1 cyc/row + FWL) or 'f32' (2x2 half-passes)
MM_DT = os.environ.get("CA_MM_DT", "bf16")
MDT = mybir.dt.bfloat16 if MM_DT == "bf16" else mybir.dt.float32


def _mm(ap):
    return ap


def build_nc(n_chunks=SQ // 128):
    nc = bacc.Bacc("TRN2", target_bir_lowering=False, debug=False)

    lat = nc.dram_tensor("latent", [SQ, D], F32, kind="ExternalInput").ap()
    ctx_d = nc.dram_tensor("context", [SKV, DC], F32, kind="ExternalInput").ap()
    wq = nc.dram_tensor("wq", [D, D], F32, kind="ExternalInput").ap()
    bq = nc.dram_tensor("bq", [D], F32, kind="ExternalInput").ap()
    wk = nc.dram_tensor("wk", [DC, D], F32, kind="ExternalInput").ap()
    bk = nc.dram_tensor("bk", [D], F32, kind="ExternalInput").ap()
    wv = nc.dram_tensor("wv", [DC, D], F32, kind="ExternalInput").ap()
    bv = nc.dram_tensor("bv", [D], F32, kind="ExternalInput").ap()
    wo = nc.dram_tensor("wo", [D, D], F32, kind="ExternalInput").ap()
    bo = nc.dram_tensor("bo", [D], F32, kind="ExternalInput").ap()
    out_d = nc.dram_tensor("out", [SQ, D], F32, kind="ExternalOutput").ap()

    with tile.TileContext(nc) as tc:
        with ExitStack() as stk:
            consts = stk.enter_context(tc.tile_pool(name="consts", bufs=1))
            prep = stk.enter_context(tc.tile_pool(name="prep", bufs=1))
            xpool = stk.enter_context(tc.tile_pool(name="x", bufs=4))
            spool = stk.enter_context(tc.tile_pool(name="work", bufs=4))
            opool = stk.enter_context(tc.tile_pool(name="outp", bufs=4))
            pspool = stk.enter_context(
                tc.tile_pool(name="psA", bufs=3, space="PSUM")
            )
            psB = stk.enter_context(
                tc.tile_pool(name="psB", bufs=5, space="PSUM")
            )

            def ps_tile(name):
                return psB.tile([128, 4, 128], F32, tag="psB", name=name)

            def load_w(ap_in, shape, name):
                t = consts.tile(shape, MDT, name=name)
                if MDT == F32:
                    nc.sync.dma_start(t, ap_in)
                else:
                    nc.gpsimd.dma_start(t, ap_in)  # SWDGE casts f32 -> bf16
                return t

            # ---------------- constants ----------------
            wq_sb = load_w(wq.rearrange("(t p) d -> p t d", p=128), [128, 4, D], "wq_sb")
            wk_sb = load_w(wk.rearrange("(t p) d -> p t d", p=128), [128, 6, D], "wk_sb")
            wv_sb = load_w(wv.rearrange("(t p) d -> p t d", p=128), [128, 6, D], "wv_sb")
            wo_sb = load_w(wo.rearrange("(t p) d -> p t d", p=128), [128, 4, D], "wo_sb")

            # bq in row-0-padded layout for the PE rank-1 bias matmul
            bq_pad = consts.tile([128, D], MDT, name="bq_pad")
            nc.vector.memset(bq_pad, 0.0)
            (nc.sync if MDT == F32 else nc.gpsimd).dma_start(bq_pad[0:1, :], bq.rearrange("(o d) -> o d", o=1))
            bk_hm = consts.tile([64, H], F32, name="bk_hm")
            nc.sync.dma_start(bk_hm, bk.rearrange("(h p) -> p h", p=64))
            bk_hms = consts.tile([64, H], F32, name="bk_hms")
            nc.vector.tensor_scalar_mul(bk_hms, bk_hm, 0.125)

            # row-0 padded biases for K=128 rank-1 bias matmuls
            bv_pad = consts.tile([128, D], MDT, name="bv_pad")
            nc.vector.memset(bv_pad, 0.0)
            (nc.sync if MDT == F32 else nc.gpsimd).dma_start(bv_pad[0:1, :], bv.rearrange("(o d) -> o d", o=1))
            bo_pad = consts.tile([128, D], MDT, name="bo_pad")
            nc.vector.memset(bo_pad, 0.0)
            (nc.sync if MDT == F32 else nc.gpsimd).dma_start(bo_pad[0:1, :], bo.rearrange("(o d) -> o d", o=1))
            # e0 [128,128]: row 0 all-ones, rest zero (lhsT of bias matmuls)
            e0 = consts.tile([128, 128], MDT, name="e0")
            nc.vector.memset(e0, 0.0)
            nc.vector.memset(e0[0:1, :], 1.0)
            # ones on rows < SKV, zeros below (lhsT of softmax-sum matmuls)
            ones_kv = consts.tile([128, DH], MDT, name="ones_kv")
            nc.vector.memset(ones_kv, 0.0)
            nc.vector.memset(ones_kv[:64, :], 1.0)
            nc.vector.memset(ones_kv[64:SKV, :], 1.0)
            ident = consts.tile([128, 128], MDT, name="ident")
            make_identity(nc, ident)
            zeros_sb = consts.tile([64, H, 128], MDT, name="zeros_sb")
            nc.vector.memset(zeros_sb, 0.0)

            # ---------------- K/V prep (once) ----------------
            ctx_sb = prep.tile([128, DC], MDT, name="ctx_sb")
            nc.vector.memset(ctx_sb, 0.0)
            (nc.sync if MDT == F32 else nc.gpsimd).dma_start(ctx_sb[:SKV, :], ctx_d)
            # cT zero-padded beyond kv=77 comes out of transposing zero rows
            cT_sb = prep.tile([128, 6, SKV], MDT, name="cT_sb")
            for g in range(2):
                cT_ps = psB.tile([128, 3, 128], MDT, tag="psB", name="cT_ps")
                for t3 in range(3):
                    t = g * 3 + t3
                    nc.tensor.transpose(
                        cT_ps[:, t3, :], ctx_sb[:, t * 128 : (t + 1) * 128], ident
                    )
                nc.vector.tensor_copy(cT_sb[:, 3 * g : 3 * g + 3, :], cT_ps[:, :, :SKV])

            # kT head-major, rows >= 64 zero
            kT_sb = prep.tile([128, H, SKV], MDT, name="kT_sb")
            nc.vector.memset(kT_sb[64:128, :, :], 0.0)
            for g in range(2):
                kT_ps = psB.tile([64, 4, SKV], F32, tag="psB", name="kT_ps")
                for hi in range(4):
                    h = g * 4 + hi
                    for ct in range(6):
                        nc.tensor.matmul(
                            kT_ps[:, hi, :],
                            lhsT=_mm(wk_sb[:, ct, h * 64 : (h + 1) * 64]),
                            rhs=_mm(cT_sb[:, ct, :]),
                            start=(ct == 0),
                            stop=(ct == 5),
                        )
                for hi in range(4):
                    h = g * 4 + hi
                    nc.scalar.activation(
                        kT_sb[:64, h, :],
                        kT_ps[:, hi, :],
                        AF.Identity,
                        bias=bk_hms[:, h : h + 1],
                        scale=0.125,
                    )

            # v zero-padded beyond kv=77
            v_ps = psB.tile([128, 4, 128], F32, tag="psB", name="v_ps")
            for ct in range(6):
                nc.tensor.matmul(
                    v_ps[:SKV, :, :].rearrange("p a b -> p (a b)"),
                    lhsT=_mm(cT_sb[:, ct, :]),
                    rhs=_mm(wv_sb[:, ct, :]),
                    start=(ct == 0),
                    stop=False,
                )
            nc.tensor.matmul(
                v_ps[:SKV, :, :].rearrange("p a b -> p (a b)"),
                lhsT=_mm(e0[:, :SKV]),
                rhs=_mm(bv_pad),
                start=False,
                stop=True,
            )
            v_sb = prep.tile([128, D], MDT, name="v_sb")
            nc.vector.memset(v_sb[64:128, :], 0.0)
            nc.vector.tensor_copy(
                v_sb[:SKV, :], v_ps[:SKV, :, :].rearrange("p a b -> p (a b)")
            )

            # ---------------- main loop: 2-stage software pipeline ----------
            # stageA(ci): load+transpose+project -> qT_sb; stageB(ci): softmax
            # +PV+out_proj. Emitting A(i+1) before B(i) lets the PE chew the
            # next chunk's projection while ACT/DVE run this chunk's softmax.
            stage_state = {}

            def stageA(ci):
                r0 = ci * 128
                x_sb = xpool.tile([128, D], MDT, tag="x", name="x_sb")
                (nc.sync if MDT == F32 else nc.gpsimd).dma_start(x_sb, lat[r0 : r0 + 128, :])

                xT_ps = pspool.tile([128, 4, 128], MDT, tag="psA", name="xT_ps")
                for et in range(4):
                    nc.tensor.transpose(
                        xT_ps[:, et, :], x_sb[:, et * 128 : (et + 1) * 128], ident
                    )
                xT_sb = spool.tile([128, 4, 128], MDT, tag="xT", name="xT_sb")
                nc.scalar.copy(xT_sb, xT_ps)

                # qT head-major [64, 8, 128] in psum (two 1-bank tiles)
                qT_sb = spool.tile([128, H, 128], MDT, tag="qT", name="qT_sb")
                nc.vector.tensor_copy(qT_sb[64:128, :, :], zeros_sb)
                for g in range(2):
                    qT_ps = pspool.tile([64, 4, 128], F32, tag="psA", name="qT_ps")
                    for hi in range(4):
                        h = g * 4 + hi
                        for et in range(4):
                            nc.tensor.matmul(
                                qT_ps[:, hi, :],
                                lhsT=_mm(wq_sb[:, et, h * 64 : (h + 1) * 64]),
                                rhs=_mm(xT_sb[:, et, :]),
                                start=(et == 0),
                                stop=False,
                            )
                        nc.tensor.matmul(
                            qT_ps[:, hi, :],
                            lhsT=_mm(bq_pad[:, h * 64 : (h + 1) * 64]),
                            rhs=_mm(e0),
                            start=False,
                            stop=True,
                        )
                    nc.vector.tensor_copy(
                        qT_sb[:64, g * 4 : g * 4 + 4, :], qT_ps
                    )

                stage_state[ci] = qT_sb

            def stageB(ci):
                r0 = ci * 128
                qT_sb = stage_state.pop(ci)
                expT_sb = spool.tile([128, H, 128], MDT, tag="expT", name="expT_sb")
                nc.vector.tensor_copy(expT_sb[64:128, :, :], zeros_sb)
                for g in range(2):
                    sT_ps = ps_tile("sT_ps")
                    for hi in range(4):
                        h = g * 4 + hi
                        nc.tensor.matmul(
                            sT_ps[:SKV, hi, :],
                            lhsT=_mm(kT_sb[:, h, :]),
                            rhs=_mm(qT_sb[:, h, :]),
                            start=True,
                            stop=True,
                        )
                    nc.scalar.activation(
                        expT_sb[:SKV, g * 4 : g * 4 + 4, :],
                        sT_ps[:SKV, :, :],
                        AF.Exp,
                    )

                sums_ps = ps_tile("sums_ps")
                for h in range(H):
                    dt, off = h // 2, (h % 2) * 64
                    nc.tensor.matmul(
                        sums_ps[off : off + 64, dt, :],
                        lhsT=_mm(ones_kv),
                        rhs=_mm(expT_sb[:, h, :]),
                        start=True,
                        stop=True,
                    )
                rsum_sb = spool.tile([128, 4, 128], F32, tag="rsum", name="rsum_sb")
                nc.vector.reciprocal_approx_fast(rsum_sb, sums_ps)

                attnT_ps = ps_tile("attnT_ps")
                for h in range(H):
                    dt, off = h // 2, (h % 2) * 64
                    nc.tensor.matmul(
                        attnT_ps[off : off + 64, dt, :],
                        lhsT=_mm(v_sb[:, h * 64 : (h + 1) * 64]),
                        rhs=_mm(expT_sb[:, h, :]),
                        start=True,
                        stop=True,
                    )
                attnT_sb = spool.tile([128, 4, 128], MDT, tag="attnT", name="attnT_sb")
                nc.vector.tensor_mul(attnT_sb, attnT_ps, rsum_sb)

                out_ps = ps_tile("out_ps")
                out_flat = out_ps.rearrange("p a b -> p (a b)")
                for dt in range(4):
                    nc.tensor.matmul(
                        out_flat,
                        lhsT=_mm(attnT_sb[:, dt, :]),
                        rhs=_mm(wo_sb[:, dt, :]),
                        start=(dt == 0),
                        stop=False,
                    )
                nc.tensor.matmul(
                    out_flat, lhsT=_mm(e0), rhs=_mm(bo_pad), start=False, stop=True
                )
                out_sb = opool.tile([128, D], F32, tag="out", name="out_sb")
                nc.scalar.copy(out_sb, out_flat)
                nc.sync.dma_start(out_d[r0 : r0 + 128, :], out_sb)

            stageA(0)
            for ci in range(1, n_chunks):
                stageA(ci)
                stageB(ci - 1)
            stageB(n_chunks - 1)

    nc.compile()
    return nc


_BUILD_CACHE = {}


def _get_nc():
    key = (MM_DT,)
    if key not in _BUILD_CACHE:
        _BUILD_CACHE[key] = build_nc()
    return _BUILD_CACHE[key]


def _in_maps(latent, context, wq, bq, wk, bk, wv, bv, wo, bo):
    f = lambda a: np.ascontiguousarray(np.asarray(a), dtype=np.float32)
    shared = {
        "wq": f(wq), "bq": f(bq), "wk": f(wk), "bk": f(bk),
        "wv": f(wv), "bv": f(bv), "wo": f(wo), "bo": f(bo),
    }
    maps = []
    for b in range(N_CORES):
        m = dict(shared)
        m["latent"] = f(latent[b])
        m["context"] = f(context[b])
        maps.append(m)
    return maps


def run_on_hw(inputs, trace=False, **kw):
    nc = _get_nc()
    maps = _in_maps(**inputs)
    res = run_bass_kernel_spmd(nc, maps, list(range(N_CORES)), trace=trace, **kw)
    out = np.stack([res.results[b]["out"] for b in range(N_CORES)], axis=0)
    return out, res


def kernel(latent, context, wq, bq, wk, bk, wv, bv, wo, bo):
    out, _ = run_on_hw(dict(
        latent=latent, context=context, wq=wq, bq=bq, wk=wk, bk=bk,
        wv=wv, bv=bv, wo=wo, bo=bo,
    ))
    return out



# revision 8
# speedup vs baseline: 1.4019x; 1.4019x over previous
"""Trainium2 Bass kernel for CrossAttention.

Problem shape (hardcoded):
  latent  [8, 4096, 512], context [8, 77, 768]
  wq [512,512], wk/wv [768,512], wo [512,512], biases [512]
  out = softmax((latent@wq+bq)(context@wk+bk)^T / 8) @ (context@wv+bv) @ wo + bo

Sharding: data-parallel over batch — core b handles batch element b.

Perf design notes (v2, from baseline trace analysis):
  The v1 baseline was LDWEIGHTS-bound: ~2400 matmuls with 64/77-col
  stationary weights -> no FWL, ~110ns weight load per MM, PE weight path
  saturated at 107% of span. v2 restructures to N=512 streaming matmuls
  with 128-col bf16 weights (FWL + hidden under the 213ns streams):

  one-time prep:
    cT    = context^T                  [128, 6, 77]  bf16 (zero-padded kv)
    kT2   = 0.125*(wk^T cT + bk)       [128, 8, 77]  head-parity layout:
            head h lives in rows (h%2)*64..+64 of [:, h, :], other half 0.
    v_aug = [v_h | ones]               [128, 8, 128] cols 0:64 = V_h rows
            (kv-padded with zeros), cols 64:128 = ones over kv rows.
  per 512-row block (8 blocks):
    xT    = x^T                        [128, 4, 512] (16 PE transposes)
    qT    = wq^T xT (+bq via ACT)      [128, 4, 512] head-pair-major
    sT_h  = kT2_h^T qT_pair            [77, 512]  (zero rows select head)
    eT_h  = exp(sT_h)                  [128, 8, 512] (pad rows memset once)
    pv_h  = v_aug_h^T eT_h             [128, 512]: rows 0:64 = attn^T,
            rows 64:128 = sum_kv(exp) replicated (ones columns)
    oT_h  = pv[0:64] / pv[64:128]      one DVE divide, no broadcast needed
    out   = oT^T wo + bo (rank-1 MM)   [512, 512] -> DMA
"""

import os
import sys
from contextlib import ExitStack

import numpy as np

for _p in ("/opt/trn_rl_repo",):
    if _p not in sys.path and os.path.isdir(_p):
        sys.path.insert(0, _p)

import concourse.bass as bass  # noqa: E402
import concourse.tile as tile  # noqa: E402
from concourse import bacc, mybir  # noqa: E402
from concourse.bass_utils import run_bass_kernel_spmd  # noqa: E402
from concourse.masks import make_identity  # noqa: E402

N_CORES = 8
MM_DT = "bf16"  # informational (test.py prints it)
SQ, D, DC, SKV, H, DH = 4096, 512, 768, 77, 8, 64
F32 = mybir.dt.float32
BF16 = mybir.dt.bfloat16
AF = mybir.ActivationFunctionType
ALU = mybir.AluOpType

N_BLOCKS = SQ // 512  # 8 blocks of 512 query rows


def build_nc():
    nc = bacc.Bacc("TRN2", target_bir_lowering=False, debug=False)

    lat = nc.dram_tensor("latent", [SQ, D], F32, kind="ExternalInput").ap()
    ctx_d = nc.dram_tensor("context", [SKV, DC], F32, kind="ExternalInput").ap()
    wq = nc.dram_tensor("wq", [D, D], F32, kind="ExternalInput").ap()
    bq = nc.dram_tensor("bq", [D], F32, kind="ExternalInput").ap()
    wk = nc.dram_tensor("wk", [DC, D], F32, kind="ExternalInput").ap()
    bk = nc.dram_tensor("bk", [D], F32, kind="ExternalInput").ap()
    wv = nc.dram_tensor("wv", [DC, D], F32, kind="ExternalInput").ap()
    bv = nc.dram_tensor("bv", [D], F32, kind="ExternalInput").ap()
    wo = nc.dram_tensor("wo", [D, D], F32, kind="ExternalInput").ap()
    bo = nc.dram_tensor("bo", [D], F32, kind="ExternalInput").ap()
    out_d = nc.dram_tensor("out", [SQ, D], F32, kind="ExternalOutput").ap()

    with tile.TileContext(nc) as tc:
        with ExitStack() as stk:
            consts = stk.enter_context(tc.tile_pool(name="consts", bufs=1))
            xpool = stk.enter_context(tc.tile_pool(name="x", bufs=3))
            xtp = stk.enter_context(tc.tile_pool(name="xt", bufs=2))
            qtp = stk.enter_context(tc.tile_pool(name="qt", bufs=2))
            otp = stk.enter_context(tc.tile_pool(name="ot", bufs=2))
            outp = stk.enter_context(tc.tile_pool(name="outp", bufs=3))
            # PSUM: 8 banks total. trq serves transposes+q-proj, spv serves
            # scores+pv, op serves out-proj.
            trq = stk.enter_context(tc.tile_pool(name="trq", bufs=2, space="PSUM"))
            spv = stk.enter_context(tc.tile_pool(name="spv", bufs=4, space="PSUM"))
            op = stk.enter_context(tc.tile_pool(name="op", bufs=2, space="PSUM"))

            def loadw(ap_in, shape, name):
                t = consts.tile(shape, BF16, name=name)
                nc.gpsimd.dma_start(t, ap_in)  # SWDGE casts f32 -> bf16
                return t

            # ---------------- constants ----------------
            wq_sb = loadw(wq.rearrange("(t p) d -> p t d", p=128), [128, 4, D], "wq_sb")
            wk_sb = loadw(wk.rearrange("(t p) d -> p t d", p=128), [128, 6, D], "wk_sb")
            wv_sb = loadw(wv.rearrange("(t p) d -> p t d", p=128), [128, 6, D], "wv_sb")
            wo_sb = loadw(wo.rearrange("(t p) d -> p t d", p=128), [128, 4, D], "wo_sb")

            bq_sb = consts.tile([128, 4], F32, name="bq_sb")
            nc.sync.dma_start(bq_sb, bq.rearrange("(t p) -> p t", p=128))
            bk_sb = consts.tile([128, 4], F32, name="bk_sb")
            nc.sync.dma_start(bk_sb, bk.rearrange("(t p) -> p t", p=128))
            bk_s = consts.tile([128, 4], F32, name="bk_s")
            nc.vector.tensor_scalar_mul(bk_s, bk_sb, 0.125)

            # row-0 padded biases for rank-1 bias matmuls
            bv_pad = consts.tile([128, D], BF16, name="bv_pad")
            nc.vector.memset(bv_pad, 0.0)
            nc.gpsimd.dma_start(bv_pad[0:1, :], bv.rearrange("(o d) -> o d", o=1))
            bo_pad = consts.tile([128, D], BF16, name="bo_pad")
            nc.vector.memset(bo_pad, 0.0)
            nc.gpsimd.dma_start(bo_pad[0:1, :], bo.rearrange("(o d) -> o d", o=1))
            # e0 [128,128]: row 0 all-ones, rest zero (lhsT of bias matmuls)
            e0 = consts.tile([128, 128], BF16, name="e0")
            nc.vector.memset(e0, 0.0)
            nc.vector.memset(e0[0:1, :], 1.0)
            ident = consts.tile([128, 128], BF16, name="ident")
            make_identity(nc, ident)

            # ---------------- K/V prep (once) ----------------
            ctx_sb = consts.tile([128, DC], BF16, name="ctx_sb")
            nc.vector.memset(ctx_sb, 0.0)
            nc.gpsimd.dma_start(ctx_sb[:SKV, :], ctx_d)
            # cT zero-padded beyond kv=77 comes from transposing zero rows
            cT_sb = consts.tile([128, 6, SKV], BF16, name="cT_sb")
            for g in range(2):
                cT_ps = trq.tile([128, 3, 128], BF16, tag="trq", name="cT_ps")
                for t3 in range(3):
                    t = g * 3 + t3
                    nc.tensor.transpose(
                        cT_ps[:, t3, :], ctx_sb[:, t * 128 : (t + 1) * 128], ident
                    )
                nc.vector.tensor_copy(cT_sb[:, 3 * g : 3 * g + 3, :], cT_ps[:, :, :SKV])

            # kT2 [128, 8, 77]: head h in rows (h%2)*64..+64 of [:, h, :],
            # other 64 rows zero -> a single matmul against the full 128-row
            # head-pair qT tile selects head h (zero rows kill the other head).
            kT2 = consts.tile([128, H, SKV], BF16, name="kT2")
            nc.vector.memset(kT2, 0.0)
            for t in range(4):  # head pair t: heads 2t (top), 2t+1 (bottom)
                kT_ps = spv.tile([128, SKV], F32, tag="spv", name="kT_ps")
                for ct in range(6):
                    nc.tensor.matmul(
                        kT_ps,
                        lhsT=wk_sb[:, ct, t * 128 : (t + 1) * 128],
                        rhs=cT_sb[:, ct, :],
                        start=(ct == 0),
                        stop=(ct == 5),
                    )
                nc.scalar.activation(
                    kT2[0:64, 2 * t, :], kT_ps[0:64, :], AF.Identity,
                    bias=bk_s[0:64, t : t + 1], scale=0.125,
                )
                nc.scalar.activation(
                    kT2[64:128, 2 * t + 1, :], kT_ps[64:128, :], AF.Identity,
                    bias=bk_s[64:128, t : t + 1], scale=0.125,
                )

            # v_hm [128, 8, 64]: V per head, kv on partitions (zero-padded
            # beyond 77). ones_kv [128, 64]: rows 0:77 ones — lhsT of the
            # softmax-denominator matmuls (all 64 output rows = the sum).
            v_ps = spv.tile([SKV, D], F32, tag="spv", name="v_ps")
            for ct in range(6):
                nc.tensor.matmul(
                    v_ps,
                    lhsT=cT_sb[:, ct, :],
                    rhs=wv_sb[:, ct, :],
                    start=(ct == 0),
                    stop=False,
                )
            nc.tensor.matmul(
                v_ps, lhsT=e0[:, :SKV], rhs=bv_pad, start=False, stop=True
            )
            v_hm = consts.tile([128, H, 64], BF16, name="v_hm")
            nc.vector.memset(v_hm, 0.0)
            for h in range(H):
                nc.vector.tensor_copy(
                    v_hm[0:SKV, h, :], v_ps[:, h * 64 : (h + 1) * 64]
                )
            ones_kv = consts.tile([128, 64], BF16, name="ones_kv")
            nc.vector.memset(ones_kv, 0.0)
            nc.vector.memset(ones_kv[0:64, :], 1.0)
            nc.vector.memset(ones_kv[64:SKV, :], 1.0)

            # eT buffers: rows 77:128 are read by the PV matmul (against zero
            # lhsT rows) but never written by exp — memset once so they hold
            # finite values (0 * NaN would poison the accumulation).
            eT_bufs = []
            for i in range(2):
                eT = consts.tile([128, H, 512], BF16, name=f"eT{i}")
                nc.vector.memset(eT[64:128, :, :], 0.0)
                eT_bufs.append(eT)

            # ---------------- main loop ----------------
            for bi in range(N_BLOCKS):
                r0 = bi * 512
                eT = eT_bufs[bi % 2]

                x_sb = xpool.tile([128, 4, D], BF16, tag="x", name="x_sb")
                nc.gpsimd.dma_start(
                    x_sb, lat[r0 : r0 + 512, :].rearrange("(g p) d -> p g d", p=128)
                )

                # xT [128, 4(d-chunk), 512(rows)]
                xT_sb = xtp.tile([128, 4, 512], BF16, tag="xT", name="xT_sb")
                for g in range(4):
                    tr_ps = trq.tile([128, 4, 128], BF16, tag="trq", name="tr_ps")
                    for et in range(4):
                        nc.tensor.transpose(
                            tr_ps[:, et, :], x_sb[:, g, et * 128 : (et + 1) * 128],
                            ident,
                        )
                    nc.vector.tensor_copy(
                        xT_sb[:, :, g * 128 : (g + 1) * 128], tr_ps
                    )

                # qT [128, 4(head pair), 512]; bq added via ACT per-partition
                qT_sb = qtp.tile([128, 4, 512], BF16, tag="qT", name="qT_sb")
                for m in range(4):
                    q_ps = trq.tile([128, 512], F32, tag="trq", name="q_ps")
                    for k in range(4):
                        nc.tensor.matmul(
                            q_ps,
                            lhsT=wq_sb[:, k, m * 128 : (m + 1) * 128],
                            rhs=xT_sb[:, k, :],
                            start=(k == 0),
                            stop=(k == 3),
                        )
                    nc.scalar.activation(
                        qT_sb[:, m, :], q_ps, AF.Identity,
                        bias=bq_sb[:, m : m + 1],
                    )

                # attention: scores -> exp per head; PV + sums stacked per
                # head PAIR (even head rows 0:64, odd rows 64:128) so the
                # reciprocal+multiply run as full-tile base-0 DVE ops
                # (custom DVE ops mishandle base-partition-64 inputs, and
                # tensor_tensor inputs must share partitions).
                oT_sb = otp.tile([128, 4, 512], BF16, tag="oT", name="oT_sb")
                for t in range(4):
                    for hh in range(2):
                        h = 2 * t + hh
                        s_ps = spv.tile([SKV, 512], F32, tag="spv", name="s_ps")
                        nc.tensor.matmul(
                            s_ps, lhsT=kT2[:, h, :], rhs=qT_sb[:, t, :],
                            start=True, stop=True,
                        )
                        nc.scalar.activation(eT[0:SKV, h, :], s_ps, AF.Exp)

                    pv_ps = spv.tile([128, 512], F32, tag="spv", name="pv_ps")
                    sm_ps = spv.tile([128, 512], F32, tag="spv", name="sm_ps")
                    for hh in range(2):
                        h = 2 * t + hh
                        o = hh * 64
                        nc.tensor.matmul(
                            pv_ps[o : o + 64, :], lhsT=v_hm[:, h, :],
                            rhs=eT[:, h, :], start=True, stop=True,
                        )
                        nc.tensor.matmul(
                            sm_ps[o : o + 64, :], lhsT=ones_kv,
                            rhs=eT[:, h, :], start=True, stop=True,
                        )
                    rs = outp.tile([128, 512], F32, tag="rs", name="rs")
                    nc.vector.reciprocal_approx_fast(rs, sm_ps)
                    nc.vector.tensor_tensor(
                        oT_sb[:, t, :], pv_ps, rs, ALU.mult
                    )

                # out projection + bo (rank-1), evacuate, store
                for r in range(4):
                    o_ps = op.tile([128, 512], F32, tag="op", name="o_ps")
                    for t in range(4):
                        nc.tensor.matmul(
                            o_ps,
                            lhsT=oT_sb[:, t, r * 128 : (r + 1) * 128],
                            rhs=wo_sb[:, t, :],
                            start=(t == 0),
                            stop=False,
                        )
                    nc.tensor.matmul(
                        o_ps, lhsT=e0, rhs=bo_pad, start=False, stop=True
                    )
                    out_sb = outp.tile([128, D], F32, tag="out", name="out_sb")
                    nc.vector.tensor_copy(out_sb, o_ps)
                    rr = r0 + r * 128
                    nc.sync.dma_start(out_d[rr : rr + 128, :], out_sb)

    nc.compile()
    return nc


_BUILD_CACHE = {}


def _get_nc():
    if "nc" not in _BUILD_CACHE:
        _BUILD_CACHE["nc"] = build_nc()
    return _BUILD_CACHE["nc"]


def _in_maps(latent, context, wq, bq, wk, bk, wv, bv, wo, bo):
    f = lambda a: np.ascontiguousarray(np.asarray(a), dtype=np.float32)
    shared = {
        "wq": f(wq), "bq": f(bq), "wk": f(wk), "bk": f(bk),
        "wv": f(wv), "bv": f(bv), "wo": f(wo), "bo": f(bo),
    }
    maps = []
    for b in range(N_CORES):
        m = dict(shared)
        m["latent"] = f(latent[b])
        m["context"] = f(context[b])
        maps.append(m)
    return maps


def run_on_hw(inputs, trace=False, **kw):
    nc = _get_nc()
    maps = _in_maps(**inputs)
    res = run_bass_kernel_spmd(nc, maps, list(range(N_CORES)), trace=trace, **kw)
    out = np.stack([res.results[b]["out"] for b in range(N_CORES)], axis=0)
    return out, res


def kernel(latent, context, wq, bq, wk, bk, wv, bv, wo, bo):
    out, _ = run_on_hw(dict(
        latent=latent, context=context, wq=wq, bq=bq, wk=wk, bk=bk,
        wv=wv, bv=bv, wo=wo, bo=bo,
    ))
    return out


# revision 9
# speedup vs baseline: 1.5053x; 1.0738x over previous
"""Trainium2 Bass kernel for CrossAttention.

Problem shape (hardcoded):
  latent  [8, 4096, 512], context [8, 77, 768]
  wq [512,512], wk/wv [768,512], wo [512,512], biases [512]
  out = softmax((latent@wq+bq)(context@wk+bk)^T / 8) @ (context@wv+bv) @ wo + bo

Sharding: data-parallel over batch — core b handles batch element b.

Perf design (v4), from trace analysis of earlier versions:
  - v1 was LDWEIGHTS-bound (~2400 small matmuls, 64/77-col weights, no FWL).
  - v2/v3 restructured to N=512 streaming matmuls: steady-state MM pitch
    hits the theoretical 216ns (512/2.4GHz + NX).
  - v4 adds: software pipelining (stage A of block b+1 interleaved into
    stage B of block b so the in-order PE queue always has issuable work),
    startup reordering (x block 0 + wq DMAs dispatched first; block-0
    transposes/Q-proj emitted before K/V prep to warm the PE HAM clock
    early), and out-proj bias via a precomputed broadcast tile instead of
    4 rank-1 matmuls per block.

Dataflow per core (one batch element), bf16 matmul operands:
  prep: cT = ctx^T; kT2 [128,8,77] head-parity layout (head h in rows
        (h%2)*64..+64, other half zero -> one matmul against the full
        128-row head-pair qT tile selects head h); v_hm [128,8,64] V per
        head kv-major; ones_kv (softmax-sum lhsT); bo_bcast [128,512].
  per 512-row block:
    xT = x^T (16 PE transposes), qT = wq^T xT + bq  [128,4,512] bf16
    sT_h = kT2_h^T qT_pair -> [77,512] psum; eT_h = exp(sT_h) (ACT)
    pv pair: attn^T stacked [even head rows 0:64 | odd 64:128], sums
        pair likewise via ones_kv -> full-tile reciprocal+multiply on DVE
        (all base-partition-0: custom DVE ops mishandle base-64 inputs)
    out = oT^T wo (+bo via DVE add) -> DMA
"""

import os
import sys
from contextlib import ExitStack

import numpy as np

for _p in ("/opt/trn_rl_repo",):
    if _p not in sys.path and os.path.isdir(_p):
        sys.path.insert(0, _p)

import concourse.bass as bass  # noqa: E402
import concourse.tile as tile  # noqa: E402
from concourse import bacc, mybir  # noqa: E402
from concourse.bass_utils import run_bass_kernel_spmd  # noqa: E402
from concourse.masks import make_identity  # noqa: E402

N_CORES = 8
MM_DT = "bf16"  # informational (test.py prints it)
SQ, D, DC, SKV, H, DH = 4096, 512, 768, 77, 8, 64
F32 = mybir.dt.float32
BF16 = mybir.dt.bfloat16
AF = mybir.ActivationFunctionType
ALU = mybir.AluOpType

N_BLOCKS = SQ // 512  # 8 blocks of 512 query rows


def build_nc():
    nc = bacc.Bacc("TRN2", target_bir_lowering=False, debug=False)

    lat = nc.dram_tensor("latent", [SQ, D], F32, kind="ExternalInput").ap()
    ctx_d = nc.dram_tensor("context", [SKV, DC], F32, kind="ExternalInput").ap()
    wq = nc.dram_tensor("wq", [D, D], F32, kind="ExternalInput").ap()
    bq = nc.dram_tensor("bq", [D], F32, kind="ExternalInput").ap()
    wk = nc.dram_tensor("wk", [DC, D], F32, kind="ExternalInput").ap()
    bk = nc.dram_tensor("bk", [D], F32, kind="ExternalInput").ap()
    wv = nc.dram_tensor("wv", [DC, D], F32, kind="ExternalInput").ap()
    bv = nc.dram_tensor("bv", [D], F32, kind="ExternalInput").ap()
    wo = nc.dram_tensor("wo", [D, D], F32, kind="ExternalInput").ap()
    bo = nc.dram_tensor("bo", [D], F32, kind="ExternalInput").ap()
    out_d = nc.dram_tensor("out", [SQ, D], F32, kind="ExternalOutput").ap()

    with tile.TileContext(nc) as tc:
        with ExitStack() as stk:
            consts = stk.enter_context(tc.tile_pool(name="consts", bufs=1))
            xpool = stk.enter_context(tc.tile_pool(name="x", bufs=3))
            xtp = stk.enter_context(tc.tile_pool(name="xt", bufs=2))
            qtp = stk.enter_context(tc.tile_pool(name="qt", bufs=2))
            otp = stk.enter_context(tc.tile_pool(name="ot", bufs=2))
            rsp = stk.enter_context(tc.tile_pool(name="rs", bufs=2))
            outp = stk.enter_context(tc.tile_pool(name="outp", bufs=3))
            # PSUM: 8 banks. trq: transposes + q-proj (2); spv: scores +
            # pv/sums (4); op: out-proj (2).
            trq = stk.enter_context(tc.tile_pool(name="trq", bufs=2, space="PSUM"))
            spv = stk.enter_context(tc.tile_pool(name="spv", bufs=4, space="PSUM"))
            op = stk.enter_context(tc.tile_pool(name="op", bufs=2, space="PSUM"))

            # ---------- DMA dispatch order matters: x0 + wq first ----------
            x_tiles = {}

            def load_x(bi):
                if bi >= N_BLOCKS:
                    return
                t = xpool.tile([128, 4, D], BF16, tag="x", name=f"x{bi}")
                nc.gpsimd.dma_start(
                    t, lat[bi * 512 : (bi + 1) * 512, :].rearrange(
                        "(g p) d -> p g d", p=128
                    )
                )
                x_tiles[bi] = t

            def loadw(ap_in, shape, name):
                t = consts.tile(shape, BF16, name=name)
                nc.gpsimd.dma_start(t, ap_in)
                return t

            load_x(0)
            wq_sb = loadw(wq.rearrange("(t p) d -> p t d", p=128), [128, 4, D], "wq_sb")
            ctx_sb = consts.tile([128, DC], BF16, name="ctx_sb")
            nc.vector.memset(ctx_sb, 0.0)
            nc.gpsimd.dma_start(ctx_sb[:SKV, :], ctx_d)
            wk_sb = loadw(wk.rearrange("(t p) d -> p t d", p=128), [128, 6, D], "wk_sb")
            wv_sb = loadw(wv.rearrange("(t p) d -> p t d", p=128), [128, 6, D], "wv_sb")
            load_x(1)
            wo_sb = loadw(wo.rearrange("(t p) d -> p t d", p=128), [128, 4, D], "wo_sb")

            bq_sb = consts.tile([128, 4], F32, name="bq_sb")
            nc.sync.dma_start(bq_sb, bq.rearrange("(t p) -> p t", p=128))
            bk_sb = consts.tile([128, 4], F32, name="bk_sb")
            nc.sync.dma_start(bk_sb, bk.rearrange("(t p) -> p t", p=128))
            bk_s = consts.tile([128, 4], F32, name="bk_s")
            nc.vector.tensor_scalar_mul(bk_s, bk_sb, 0.125)

            bv_pad = consts.tile([128, D], BF16, name="bv_pad")
            nc.vector.memset(bv_pad, 0.0)
            nc.gpsimd.dma_start(bv_pad[0:1, :], bv.rearrange("(o d) -> o d", o=1))
            bo_pad = consts.tile([128, D], BF16, name="bo_pad")
            nc.vector.memset(bo_pad, 0.0)
            nc.gpsimd.dma_start(bo_pad[0:1, :], bo.rearrange("(o d) -> o d", o=1))
            e0 = consts.tile([128, 128], BF16, name="e0")
            nc.vector.memset(e0, 0.0)
            nc.vector.memset(e0[0:1, :], 1.0)
            ident = consts.tile([128, 128], BF16, name="ident")
            make_identity(nc, ident)

            eT_bufs = []
            for i in range(2):
                eT = consts.tile([128, H, 512], BF16, name=f"eT{i}")
                nc.vector.memset(eT[64:128, :, :], 0.0)
                eT_bufs.append(eT)

            # ---------- stage A pieces (emitted fine-grained) ----------
            xT_tiles, qT_tiles = {}, {}

            def stageA_tr(bi):
                """x^T via 16 PE transposes -> xT [128, 4(dchunk), 512]."""
                if bi >= N_BLOCKS:
                    return
                x_sb = x_tiles.pop(bi)
                xT_sb = xtp.tile([128, 4, 512], BF16, tag="xT", name="xT_sb")
                for g in range(4):
                    tr_ps = trq.tile([128, 4, 128], BF16, tag="trq", name="tr_ps")
                    for et in range(4):
                        nc.tensor.transpose(
                            tr_ps[:, et, :],
                            x_sb[:, g, et * 128 : (et + 1) * 128],
                            ident,
                        )
                    nc.vector.tensor_copy(
                        xT_sb[:, :, g * 128 : (g + 1) * 128], tr_ps
                    )
                xT_tiles[bi] = xT_sb

            def stageA_q(bi, ms):
                """Q-proj m-chunks: qT[:, m, :] = wq_m^T xT + bq (ACT bias)."""
                if bi >= N_BLOCKS:
                    return
                if bi not in qT_tiles:
                    qT_tiles[bi] = qtp.tile([128, 4, 512], BF16, tag="qT", name="qT_sb")
                xT_sb, qT_sb = xT_tiles[bi], qT_tiles[bi]
                for m in ms:
                    q_ps = trq.tile([128, 512], F32, tag="trq", name="q_ps")
                    for k in range(4):
                        nc.tensor.matmul(
                            q_ps,
                            lhsT=wq_sb[:, k, m * 128 : (m + 1) * 128],
                            rhs=xT_sb[:, k, :],
                            start=(k == 0),
                            stop=(k == 3),
                        )
                    nc.scalar.activation(
                        qT_sb[:, m, :], q_ps, AF.Identity,
                        bias=bq_sb[:, m : m + 1],
                    )
                if ms[-1] == 3:
                    xT_tiles.pop(bi)

            # ---------- block 0 stage A before K/V prep (HAM warmup, x0
            # and wq arrive first) ----------
            stageA_tr(0)
            stageA_q(0, [0, 1, 2, 3])

            # ---------- K/V prep ----------
            cT_sb = consts.tile([128, 6, SKV], BF16, name="cT_sb")
            for g in range(2):
                cT_ps = trq.tile([128, 3, 128], BF16, tag="trq", name="cT_ps")
                for t3 in range(3):
                    t = g * 3 + t3
                    nc.tensor.transpose(
                        cT_ps[:, t3, :], ctx_sb[:, t * 128 : (t + 1) * 128], ident
                    )
                nc.vector.tensor_copy(cT_sb[:, 3 * g : 3 * g + 3, :], cT_ps[:, :, :SKV])

            kT2 = consts.tile([128, H, SKV], BF16, name="kT2")
            nc.vector.memset(kT2, 0.0)
            for t in range(4):
                kT_ps = spv.tile([128, SKV], F32, tag="spv", name="kT_ps")
                for ct in range(6):
                    nc.tensor.matmul(
                        kT_ps,
                        lhsT=wk_sb[:, ct, t * 128 : (t + 1) * 128],
                        rhs=cT_sb[:, ct, :],
                        start=(ct == 0),
                        stop=(ct == 5),
                    )
                nc.scalar.activation(
                    kT2[0:64, 2 * t, :], kT_ps[0:64, :], AF.Identity,
                    bias=bk_s[0:64, t : t + 1], scale=0.125,
                )
                nc.scalar.activation(
                    kT2[64:128, 2 * t + 1, :], kT_ps[64:128, :], AF.Identity,
                    bias=bk_s[64:128, t : t + 1], scale=0.125,
                )

            v_ps = spv.tile([SKV, D], F32, tag="spv", name="v_ps")
            for ct in range(6):
                nc.tensor.matmul(
                    v_ps, lhsT=cT_sb[:, ct, :], rhs=wv_sb[:, ct, :],
                    start=(ct == 0), stop=False,
                )
            nc.tensor.matmul(
                v_ps, lhsT=e0[:, :SKV], rhs=bv_pad, start=False, stop=True
            )
            v_hm = consts.tile([128, H, 64], BF16, name="v_hm")
            nc.vector.memset(v_hm, 0.0)
            for h in range(H):
                nc.vector.tensor_copy(
                    v_hm[0:SKV, h, :], v_ps[:, h * 64 : (h + 1) * 64]
                )
            ones_kv = consts.tile([128, 64], BF16, name="ones_kv")
            nc.vector.memset(ones_kv, 0.0)
            nc.vector.memset(ones_kv[0:64, :], 1.0)
            nc.vector.memset(ones_kv[64:SKV, :], 1.0)

            # bo broadcast to all 128 partitions via one rank-1 matmul
            bo_ps = op.tile([128, 512], F32, tag="op", name="bo_ps")
            nc.tensor.matmul(bo_ps, lhsT=e0, rhs=bo_pad, start=True, stop=True)
            bo_bc = consts.tile([128, D], F32, name="bo_bc")
            nc.vector.tensor_copy(bo_bc, bo_ps)

            # ---------- main loop: stage B(bi) with stage A(bi+1)
            # interleaved for PE fill ----------
            for bi in range(N_BLOCKS):
                r0 = bi * 512
                eT = eT_bufs[bi % 2]
                qT_sb = qT_tiles.pop(bi)

                load_x(bi + 2)

                def sT(h):
                    s_ps = spv.tile([SKV, 512], F32, tag="spv", name="s_ps")
                    nc.tensor.matmul(
                        s_ps, lhsT=kT2[:, h, :], rhs=qT_sb[:, h // 2, :],
                        start=True, stop=True,
                    )
                    nc.scalar.activation(eT[0:SKV, h, :], s_ps, AF.Exp)

                # scores interleaved with next block's transposes/Q-proj
                sT(0); sT(1); sT(2); sT(3)
                stageA_tr(bi + 1)
                stageA_q(bi + 1, [0, 1])
                sT(4); sT(5); sT(6); sT(7)
                stageA_q(bi + 1, [2, 3])

                # PV + sums per head pair, then full-tile recip * mult
                oT_sb = otp.tile([128, 4, 512], BF16, tag="oT", name="oT_sb")
                for t in range(4):
                    pv_ps = spv.tile([128, 512], F32, tag="spv", name="pv_ps")
                    sm_ps = spv.tile([128, 512], F32, tag="spv", name="sm_ps")
                    for hh in range(2):
                        h = 2 * t + hh
                        o = hh * 64
                        nc.tensor.matmul(
                            pv_ps[o : o + 64, :], lhsT=v_hm[:, h, :],
                            rhs=eT[:, h, :], start=True, stop=True,
                        )
                        nc.tensor.matmul(
                            sm_ps[o : o + 64, :], lhsT=ones_kv,
                            rhs=eT[:, h, :], start=True, stop=True,
                        )
                    rs = rsp.tile([128, 512], F32, tag="rs", name="rs")
                    nc.vector.reciprocal_approx_fast(rs, sm_ps)
                    nc.vector.tensor_tensor(oT_sb[:, t, :], pv_ps, rs, ALU.mult)

                # out projection; bias added during psum evacuation
                for r in range(4):
                    o_ps = op.tile([128, 512], F32, tag="op", name="o_ps")
                    for t in range(4):
                        nc.tensor.matmul(
                            o_ps,
                            lhsT=oT_sb[:, t, r * 128 : (r + 1) * 128],
                            rhs=wo_sb[:, t, :],
                            start=(t == 0),
                            stop=(t == 3),
                        )
                    out_sb = outp.tile([128, D], F32, tag="out", name="out_sb")
                    nc.vector.tensor_tensor(out_sb, o_ps, bo_bc, ALU.add)
                    rr = r0 + r * 128
                    nc.sync.dma_start(out_d[rr : rr + 128, :], out_sb)

    nc.compile()
    return nc


_BUILD_CACHE = {}


def _get_nc():
    if "nc" not in _BUILD_CACHE:
        _BUILD_CACHE["nc"] = build_nc()
    return _BUILD_CACHE["nc"]


def _in_maps(latent, context, wq, bq, wk, bk, wv, bv, wo, bo):
    f = lambda a: np.ascontiguousarray(np.asarray(a), dtype=np.float32)
    shared = {
        "wq": f(wq), "bq": f(bq), "wk": f(wk), "bk": f(bk),
        "wv": f(wv), "bv": f(bv), "wo": f(wo), "bo": f(bo),
    }
    maps = []
    for b in range(N_CORES):
        m = dict(shared)
        m["latent"] = f(latent[b])
        m["context"] = f(context[b])
        maps.append(m)
    return maps


def run_on_hw(inputs, trace=False, **kw):
    nc = _get_nc()
    maps = _in_maps(**inputs)
    res = run_bass_kernel_spmd(nc, maps, list(range(N_CORES)), trace=trace, **kw)
    out = np.stack([res.results[b]["out"] for b in range(N_CORES)], axis=0)
    return out, res


def kernel(latent, context, wq, bq, wk, bk, wv, bv, wo, bo):
    out, _ = run_on_hw(dict(
        latent=latent, context=context, wq=wq, bq=bq, wk=wk, bk=bk,
        wv=wv, bv=bv, wo=wo, bo=bo,
    ))
    return out


# revision 12
# speedup vs baseline: 1.5746x; 1.0460x over previous
"""Trainium2 Bass kernel for CrossAttention.

Problem shape (hardcoded):
  latent  [8, 4096, 512], context [8, 77, 768]
  wq [512,512], wk/wv [768,512], wo [512,512], biases [512]
  out = softmax((latent@wq+bq)(context@wk+bk)^T / 8) @ (context@wv+bv) @ wo + bo

Sharding: data-parallel over batch — core b handles batch element b.

Perf design (v4), from trace analysis of earlier versions:
  - v1 was LDWEIGHTS-bound (~2400 small matmuls, 64/77-col weights, no FWL).
  - v2/v3 restructured to N=512 streaming matmuls: steady-state MM pitch
    hits the theoretical 216ns (512/2.4GHz + NX).
  - v4 adds: software pipelining (stage A of block b+1 interleaved into
    stage B of block b so the in-order PE queue always has issuable work),
    startup reordering (x block 0 + wq DMAs dispatched first; block-0
    transposes/Q-proj emitted before K/V prep to warm the PE HAM clock
    early), and out-proj bias via a precomputed broadcast tile instead of
    4 rank-1 matmuls per block.

Dataflow per core (one batch element), bf16 matmul operands:
  prep: cT = ctx^T; kT2 [128,8,77] head-parity layout (head h in rows
        (h%2)*64..+64, other half zero -> one matmul against the full
        128-row head-pair qT tile selects head h); v_hm [128,8,64] V per
        head kv-major; ones_kv (softmax-sum lhsT); bo_bcast [128,512].
  per 512-row block:
    xT = x^T (16 PE transposes), qT = wq^T xT + bq  [128,4,512] bf16
    sT_h = kT2_h^T qT_pair -> [77,512] psum; eT_h = exp(sT_h) (ACT)
    pv pair: attn^T stacked [even head rows 0:64 | odd 64:128], sums
        pair likewise via ones_kv -> full-tile reciprocal+multiply on DVE
        (all base-partition-0: custom DVE ops mishandle base-64 inputs)
    out = oT^T wo (+bo via DVE add) -> DMA
"""

import os
import sys
from contextlib import ExitStack

import numpy as np

for _p in ("/opt/trn_rl_repo",):
    if _p not in sys.path and os.path.isdir(_p):
        sys.path.insert(0, _p)

import concourse.bass as bass  # noqa: E402
import concourse.tile as tile  # noqa: E402
from concourse import bacc, mybir  # noqa: E402
from concourse.bass_utils import run_bass_kernel_spmd  # noqa: E402
from concourse.masks import make_identity  # noqa: E402

N_CORES = 8
MM_DT = "bf16"  # informational (test.py prints it)
SQ, D, DC, SKV, H, DH = 4096, 512, 768, 77, 8, 64
F32 = mybir.dt.float32
BF16 = mybir.dt.bfloat16
AF = mybir.ActivationFunctionType
ALU = mybir.AluOpType

N_BLOCKS = SQ // 512  # 8 blocks of 512 query rows


def build_nc():
    nc = bacc.Bacc("TRN2", target_bir_lowering=False, debug=False)

    # bulk tensors arrive pre-cast to bf16 (host-side dtype choice, same
    # numerics as an on-device cast) so every load is a fast HW-DGE
    # transfer; bq/bk stay f32 (ACT bias operands must be f32).
    lat = nc.dram_tensor("latent", [SQ, D], BF16, kind="ExternalInput").ap()
    ctx_d = nc.dram_tensor("context", [SKV, DC], BF16, kind="ExternalInput").ap()
    wq = nc.dram_tensor("wq", [D, D], BF16, kind="ExternalInput").ap()
    bq = nc.dram_tensor("bq", [D], F32, kind="ExternalInput").ap()
    wk = nc.dram_tensor("wk", [DC, D], BF16, kind="ExternalInput").ap()
    bk = nc.dram_tensor("bk", [D], F32, kind="ExternalInput").ap()
    wv = nc.dram_tensor("wv", [DC, D], BF16, kind="ExternalInput").ap()
    bv = nc.dram_tensor("bv", [D], BF16, kind="ExternalInput").ap()
    wo = nc.dram_tensor("wo", [D, D], BF16, kind="ExternalInput").ap()
    bo = nc.dram_tensor("bo", [D], BF16, kind="ExternalInput").ap()
    out_d = nc.dram_tensor("out", [SQ, D], F32, kind="ExternalOutput").ap()

    with tile.TileContext(nc) as tc:
        with ExitStack() as stk:
            consts = stk.enter_context(tc.tile_pool(name="consts", bufs=1))
            xpool = stk.enter_context(tc.tile_pool(name="x", bufs=3))
            xtp = stk.enter_context(tc.tile_pool(name="xt", bufs=2))
            qtp = stk.enter_context(tc.tile_pool(name="qt", bufs=2))
            otp = stk.enter_context(tc.tile_pool(name="ot", bufs=2))
            rsp = stk.enter_context(tc.tile_pool(name="rs", bufs=2))
            outp = stk.enter_context(tc.tile_pool(name="outp", bufs=3))
            # PSUM: 8 banks. trq: transposes + q-proj (2); spv: scores +
            # pv/sums (4); op: out-proj (2).
            trq = stk.enter_context(tc.tile_pool(name="trq", bufs=2, space="PSUM"))
            spv = stk.enter_context(tc.tile_pool(name="spv", bufs=4, space="PSUM"))
            op = stk.enter_context(tc.tile_pool(name="op", bufs=2, space="PSUM"))

            # ---------- constants the PE needs first (gpsimd/DVE, instant;
            # emitted before any DMA so nothing queues behind transfers) ----
            ident = consts.tile([128, 128], BF16, name="ident")
            make_identity(nc, ident)
            e0 = consts.tile([128, 128], BF16, name="e0")
            nc.vector.memset(e0, 0.0)
            nc.vector.memset(e0[0:1, :], 1.0)
            eT_bufs = []
            for i in range(2):
                eT = consts.tile([128, H, 512], BF16, name=f"eT{i}")
                nc.vector.memset(eT[64:128, :, :], 0.0)
                eT_bufs.append(eT)

            # ---------- loads: x on the scalar HW-DGE queue, everything
            # else on sync (stores also use sync, but arrive later) ----------
            x_tiles = {}

            def load_x(bi):
                if bi >= N_BLOCKS:
                    return
                t = xpool.tile([128, 4, D], BF16, tag="x", name=f"x{bi}")
                nc.scalar.dma_start(
                    t, lat[bi * 512 : (bi + 1) * 512, :].rearrange(
                        "(g p) d -> p g d", p=128
                    )
                )
                x_tiles[bi] = t

            def loadw(ap_in, shape, name):
                t = consts.tile(shape, BF16, name=name)
                nc.sync.dma_start(t, ap_in)
                return t

            load_x(0)
            wq_sb = loadw(wq.rearrange("(t p) d -> p t d", p=128), [128, 4, D], "wq_sb")
            ctx_sb = consts.tile([128, DC], BF16, name="ctx_sb")
            nc.vector.memset(ctx_sb, 0.0)
            nc.sync.dma_start(ctx_sb[:SKV, :], ctx_d)
            wk_sb = loadw(wk.rearrange("(t p) d -> p t d", p=128), [128, 6, D], "wk_sb")
            wv_sb = loadw(wv.rearrange("(t p) d -> p t d", p=128), [128, 6, D], "wv_sb")
            load_x(1)
            wo_sb = loadw(wo.rearrange("(t p) d -> p t d", p=128), [128, 4, D], "wo_sb")

            bq_sb = consts.tile([128, 4], F32, name="bq_sb")
            nc.sync.dma_start(bq_sb, bq.rearrange("(t p) -> p t", p=128))
            bk_sb = consts.tile([128, 4], F32, name="bk_sb")
            nc.sync.dma_start(bk_sb, bk.rearrange("(t p) -> p t", p=128))
            bk_s = consts.tile([128, 4], F32, name="bk_s")
            nc.vector.tensor_scalar_mul(bk_s, bk_sb, 0.125)

            bv_pad = consts.tile([128, D], BF16, name="bv_pad")
            nc.vector.memset(bv_pad, 0.0)
            nc.sync.dma_start(bv_pad[0:1, :], bv.rearrange("(o d) -> o d", o=1))
            bo_pad = consts.tile([128, D], BF16, name="bo_pad")
            nc.vector.memset(bo_pad, 0.0)
            nc.sync.dma_start(bo_pad[0:1, :], bo.rearrange("(o d) -> o d", o=1))

            # ---------- stage A pieces (emitted fine-grained) ----------
            xT_tiles, qT_tiles = {}, {}

            def stageA_tr(bi):
                """x^T via 16 PE transposes -> xT [128, 4(dchunk), 512]."""
                if bi >= N_BLOCKS:
                    return
                x_sb = x_tiles.pop(bi)
                xT_sb = xtp.tile([128, 4, 512], BF16, tag="xT", name="xT_sb")
                for g in range(4):
                    tr_ps = trq.tile([128, 4, 128], BF16, tag="trq", name="tr_ps")
                    for et in range(4):
                        nc.tensor.transpose(
                            tr_ps[:, et, :],
                            x_sb[:, g, et * 128 : (et + 1) * 128],
                            ident,
                        )
                    nc.vector.tensor_copy(
                        xT_sb[:, :, g * 128 : (g + 1) * 128], tr_ps
                    )
                xT_tiles[bi] = xT_sb

            def stageA_q(bi, ms):
                """Q-proj m-chunks: qT[:, m, :] = wq_m^T xT + bq (ACT bias)."""
                if bi >= N_BLOCKS:
                    return
                if bi not in qT_tiles:
                    qT_tiles[bi] = qtp.tile([128, 4, 512], BF16, tag="qT", name="qT_sb")
                xT_sb, qT_sb = xT_tiles[bi], qT_tiles[bi]
                for m in ms:
                    q_ps = trq.tile([128, 512], F32, tag="trq", name="q_ps")
                    for k in range(4):
                        nc.tensor.matmul(
                            q_ps,
                            lhsT=wq_sb[:, k, m * 128 : (m + 1) * 128],
                            rhs=xT_sb[:, k, :],
                            start=(k == 0),
                            stop=(k == 3),
                        )
                    nc.scalar.activation(
                        qT_sb[:, m, :], q_ps, AF.Identity,
                        bias=bq_sb[:, m : m + 1],
                    )
                if ms[-1] == 3:
                    xT_tiles.pop(bi)

            # ---------- block 0 stage A before K/V prep (HAM warmup, x0
            # and wq arrive first) ----------
            stageA_tr(0)
            stageA_q(0, [0, 1, 2, 3])

            # ---------- K/V prep ----------
            cT_sb = consts.tile([128, 6, SKV], BF16, name="cT_sb")
            for g in range(2):
                cT_ps = trq.tile([128, 3, 128], BF16, tag="trq", name="cT_ps")
                for t3 in range(3):
                    t = g * 3 + t3
                    nc.tensor.transpose(
                        cT_ps[:, t3, :], ctx_sb[:, t * 128 : (t + 1) * 128], ident
                    )
                nc.vector.tensor_copy(cT_sb[:, 3 * g : 3 * g + 3, :], cT_ps[:, :, :SKV])

            kT2 = consts.tile([128, H, SKV], BF16, name="kT2")
            nc.vector.memset(kT2, 0.0)
            for t in range(4):
                kT_ps = spv.tile([128, SKV], F32, tag="spv", name="kT_ps")
                for ct in range(6):
                    nc.tensor.matmul(
                        kT_ps,
                        lhsT=wk_sb[:, ct, t * 128 : (t + 1) * 128],
                        rhs=cT_sb[:, ct, :],
                        start=(ct == 0),
                        stop=(ct == 5),
                    )
                nc.scalar.activation(
                    kT2[0:64, 2 * t, :], kT_ps[0:64, :], AF.Identity,
                    bias=bk_s[0:64, t : t + 1], scale=0.125,
                )
                nc.scalar.activation(
                    kT2[64:128, 2 * t + 1, :], kT_ps[64:128, :], AF.Identity,
                    bias=bk_s[64:128, t : t + 1], scale=0.125,
                )

            v_ps = spv.tile([SKV, D], F32, tag="spv", name="v_ps")
            for ct in range(6):
                nc.tensor.matmul(
                    v_ps, lhsT=cT_sb[:, ct, :], rhs=wv_sb[:, ct, :],
                    start=(ct == 0), stop=False,
                )
            nc.tensor.matmul(
                v_ps, lhsT=e0[:, :SKV], rhs=bv_pad, start=False, stop=True
            )
            v_hm = consts.tile([128, H, 64], BF16, name="v_hm")
            nc.vector.memset(v_hm, 0.0)
            for h in range(H):
                nc.vector.tensor_copy(
                    v_hm[0:SKV, h, :], v_ps[:, h * 64 : (h + 1) * 64]
                )
            ones_kv = consts.tile([128, 64], BF16, name="ones_kv")
            nc.vector.memset(ones_kv, 0.0)
            nc.vector.memset(ones_kv[0:64, :], 1.0)
            nc.vector.memset(ones_kv[64:SKV, :], 1.0)

            # bo broadcast to all 128 partitions via one rank-1 matmul
            bo_ps = op.tile([128, 512], F32, tag="op", name="bo_ps")
            nc.tensor.matmul(bo_ps, lhsT=e0, rhs=bo_pad, start=True, stop=True)
            bo_bc = consts.tile([128, D], F32, name="bo_bc")
            nc.vector.tensor_copy(bo_bc, bo_ps)

            # ---------- main loop: stage B(bi) with stage A(bi+1)
            # interleaved for PE fill ----------
            for bi in range(N_BLOCKS):
                r0 = bi * 512
                eT = eT_bufs[bi % 2]
                qT_sb = qT_tiles.pop(bi)

                load_x(bi + 2)

                def sT(h):
                    s_ps = spv.tile([SKV, 512], F32, tag="spv", name="s_ps")
                    nc.tensor.matmul(
                        s_ps, lhsT=kT2[:, h, :], rhs=qT_sb[:, h // 2, :],
                        start=True, stop=True,
                    )
                    nc.scalar.activation(eT[0:SKV, h, :], s_ps, AF.Exp)

                # scores interleaved with next block's transposes/Q-proj
                sT(0); sT(1); sT(2); sT(3)
                stageA_tr(bi + 1)
                stageA_q(bi + 1, [0, 1])
                sT(4); sT(5); sT(6); sT(7)
                stageA_q(bi + 1, [2, 3])

                # PV + sums per head pair, then full-tile recip * mult
                oT_sb = otp.tile([128, 4, 512], BF16, tag="oT", name="oT_sb")
                for t in range(4):
                    pv_ps = spv.tile([128, 512], F32, tag="spv", name="pv_ps")
                    sm_ps = spv.tile([128, 512], F32, tag="spv", name="sm_ps")
                    for hh in range(2):
                        h = 2 * t + hh
                        o = hh * 64
                        nc.tensor.matmul(
                            pv_ps[o : o + 64, :], lhsT=v_hm[:, h, :],
                            rhs=eT[:, h, :], start=True, stop=True,
                        )
                        nc.tensor.matmul(
                            sm_ps[o : o + 64, :], lhsT=ones_kv,
                            rhs=eT[:, h, :], start=True, stop=True,
                        )
                    rs = rsp.tile([128, 512], F32, tag="rs", name="rs")
                    nc.vector.reciprocal_approx_fast(rs, sm_ps)
                    nc.vector.tensor_tensor(oT_sb[:, t, :], pv_ps, rs, ALU.mult)

                # out projection; bias added during psum evacuation
                for r in range(4):
                    o_ps = op.tile([128, 512], F32, tag="op", name="o_ps")
                    for t in range(4):
                        nc.tensor.matmul(
                            o_ps,
                            lhsT=oT_sb[:, t, r * 128 : (r + 1) * 128],
                            rhs=wo_sb[:, t, :],
                            start=(t == 0),
                            stop=(t == 3),
                        )
                    out_sb = outp.tile([128, D], F32, tag="out", name="out_sb")
                    nc.vector.tensor_tensor(out_sb, o_ps, bo_bc, ALU.add)
                    rr = r0 + r * 128
                    nc.sync.dma_start(out_d[rr : rr + 128, :], out_sb)

    nc.compile()
    return nc


_BUILD_CACHE = {}


def _get_nc():
    if "nc" not in _BUILD_CACHE:
        _BUILD_CACHE["nc"] = build_nc()
    return _BUILD_CACHE["nc"]


def _in_maps(latent, context, wq, bq, wk, bk, wv, bv, wo, bo):
    import ml_dtypes

    bf = ml_dtypes.bfloat16
    f = lambda a: np.ascontiguousarray(np.asarray(a), dtype=np.float32)
    fb = lambda a: np.ascontiguousarray(
        np.asarray(a, dtype=np.float32).astype(bf)
    )
    shared = {
        "wq": fb(wq), "bq": f(bq), "wk": fb(wk), "bk": f(bk),
        "wv": fb(wv), "bv": fb(bv), "wo": fb(wo), "bo": fb(bo),
    }
    maps = []
    for b in range(N_CORES):
        m = dict(shared)
        m["latent"] = fb(latent[b])
        m["context"] = fb(context[b])
        maps.append(m)
    return maps


def run_on_hw(inputs, trace=False, **kw):
    nc = _get_nc()
    maps = _in_maps(**inputs)
    res = run_bass_kernel_spmd(nc, maps, list(range(N_CORES)), trace=trace, **kw)
    out = np.stack([res.results[b]["out"] for b in range(N_CORES)], axis=0)
    return out, res


def kernel(latent, context, wq, bq, wk, bk, wv, bv, wo, bo):
    out, _ = run_on_hw(dict(
        latent=latent, context=context, wq=wq, bq=bq, wk=wk, bk=bk,
        wv=wv, bv=bv, wo=wo, bo=bo,
    ))
    return out
